# revision 8
# baseline (speedup 1.0000x reference)
"""ComposedLoRAModel forward on 8 TRN2 NeuronCores.

Math (per layer l, sample b):
    out[l,b]   = x[l,b] @ W[l].T + (x[l,b] @ Am_b.T) @ Bs_b.T
where Am_b [32, d_in] are the top-k adapters' A rows masked per-sample and
Bs_b [32, d_out] the score-scaled B columns -- the [d_out, d_in] delta of the
reference is never materialized, only its rank-32 factors are applied.
Plus two scalar losses (mask sparsity, softmax-balance KL), computed on
device without touching the tensor engine (it stays fp32r-only end to end;
mixing plain-fp32 matmuls into the fp32r stream stalls the PE on TRN2).

Sharding: data-parallel over the 16 (l, b) pairs -> core c owns layer c//2
and samples {2*(c%2), 2*(c%2)+1}. Each core holds W[l].T resident in SBUF
(16 MB), streams x.T in 2 MB chunks, and accumulates base + low-rank lora
contributions in PSUM using float32r (full-rate fp32 tensor-engine mode,
~1.5e-4 rel err). The first x chunk is DMA'd before W so the PE starts
within ~10us; chunk-0 compute then paces W's streaming arrival.

The host side only reshapes/transposes/gathers (no FLOPs on the big
tensors); every matmul, reduction and the loss math run on device.
"""
import numpy as np

import concourse.bacc as bacc
import concourse.mybir as mybir
import concourse.tile as tile
from concourse.bass_utils import run_bass_kernel_spmd

E, L, R, DIN, DOUT, B, K, S = 16, 4, 16, 2048, 2048, 4, 2, 2048
N_CORES = 8
R2 = K * R            # 32 combined low-rank columns
SC = 256              # s-chunk width streamed per DMA
NB = DIN // 128       # 16 contraction blocks
NSC = S // SC         # 8 s-chunks
NSB = SC // 128       # 2 128-row blocks per chunk
OC = 512              # output chunk width (one PSUM bank)
NOC = DOUT // OC      # 4 output chunks

F32 = mybir.dt.float32
F32R = mybir.dt.float32r

_NC_CACHE = None
LAST_RESULT = None  # stashed BassKernelResults for external profiling


def _emit_losses(nc, pool, masks, scores, losses_dram):
    """Sparsity mean + balance KL on DVE/ACT only (no tensor engine)."""
    losses_sb = pool.tile([1, 2], F32, name="losses_sb")

    masks_sb = pool.tile([1, E * B * R], F32, name="masks_sb")
    nc.sync.dma_start(masks_sb[:], masks[:])
    msum = pool.tile([1, 1], F32, name="msum")
    nc.vector.reduce_sum(msum[:], masks_sb[:], axis=mybir.AxisListType.X)
    nc.scalar.mul(losses_sb[:, 0:1], msum[:], 1.0 / (E * B * R))

    sc_sb = pool.tile([B, E], F32, name="sc_sb")
    nc.sync.dma_start(sc_sb[:], scores[:])
    rmax = pool.tile([B, 1], F32, name="rmax")
    nc.vector.reduce_max(rmax[:], sc_sb[:], axis=mybir.AxisListType.X)
    shifted = pool.tile([B, E], F32, name="shifted")
    nc.vector.tensor_scalar_sub(shifted[:], sc_sb[:], rmax[:])
    ex = pool.tile([B, E], F32, name="ex")
    nc.scalar.activation(ex[:], shifted[:], mybir.ActivationFunctionType.Exp)
    rsum = pool.tile([B, 1], F32, name="rsum")
    nc.vector.reduce_sum(rsum[:], ex[:], axis=mybir.AxisListType.X)
    rrec = pool.tile([B, 1], F32, name="rrec")
    nc.vector.reciprocal(rrec[:], rsum[:])
    probs = pool.tile([B, E], F32, name="probs")
    nc.vector.tensor_scalar_mul(probs[:], ex[:], rrec[:])
    # flatten probs onto one partition; batch-mean via strided view reduce
    pflat = pool.tile([1, B, E], F32, name="pflat")
    nc.sync.dma_start(pflat[:], probs[:])
    mp_row = pool.tile([1, E], F32, name="mp_row")
    nc.vector.reduce_sum(mp_row[:], pflat[:].transpose([0, 2, 1]),
                         axis=mybir.AxisListType.X)
    logp = pool.tile([1, E], F32, name="logp")
    # ln(mean) = ln(sum * 1/B) via the activation pre-scale
    nc.scalar.activation(logp[:], mp_row[:], mybir.ActivationFunctionType.Ln,
                         scale=float(1.0 / B))
    u = 1.0 / E
    t_sb = pool.tile([1, E], F32, name="t_sb")
    nc.scalar.mul(t_sb[:], logp[:], float(-u / E))
    nc.vector.tensor_scalar_add(t_sb[:], t_sb[:], float(u * np.log(u) / E))
    bal = pool.tile([1, 1], F32, name="bal")
    nc.vector.reduce_sum(bal[:], t_sb[:], axis=mybir.AxisListType.X)
    nc.vector.tensor_copy(losses_sb[:, 1:2], bal[:])
    nc.sync.dma_start(losses_dram[:], losses_sb[:])


def _build(losses=True, main=True, mm_dt=F32R, nb=NB, nsc=NSC):
    nc = bacc.Bacc("TRN2", target_bir_lowering=False, debug=False,
                   num_devices=N_CORES)
    # x, pre-transposed and chunk-major on host: [b][sc][p][n][s_in_chunk]
    xT = nc.dram_tensor("xT", [2, NSC, 128, NB, SC], F32,
                        kind="ExternalInput")
    wT = nc.dram_tensor("wT", [DIN, DOUT], F32, kind="ExternalInput")
    amt = nc.dram_tensor("amt", [128, NB, 2, R2], F32, kind="ExternalInput")
    bst = nc.dram_tensor("bst", [R2, 2, DOUT], F32, kind="ExternalInput")
    masks = nc.dram_tensor("masks", [1, E * B * R], F32, kind="ExternalInput")
    scores = nc.dram_tensor("scores", [B, E], F32, kind="ExternalInput")
    out = nc.dram_tensor("out", [2, S, DOUT], F32, kind="ExternalOutput")
    losses_dram = nc.dram_tensor("losses", [1, 2], F32, kind="ExternalOutput")

    with tile.TileContext(nc) as tc:
        with (
            tc.tile_pool(name="wt", bufs=1) as wt_pool,
            tc.tile_pool(name="xq", bufs=2) as xq_pool,
            tc.tile_pool(name="small", bufs=1) as small_pool,
            tc.tile_pool(name="ut", bufs=2) as ut_pool,
            tc.tile_pool(name="osb", bufs=4) as osb_pool,
            tc.tile_pool(name="pu", bufs=2, space="PSUM") as pu_pool,
            tc.tile_pool(name="po", bufs=6, space="PSUM") as po_pool,
        ):
            # adapter factors + first x chunk first: they gate the PE start
            amt_sb = small_pool.tile([128, NB, 2, R2], mm_dt)
            nc.sync.dma_start(amt_sb[:], amt[:].bitcast(mm_dt))
            bst_sb = small_pool.tile([R2, 2, DOUT], mm_dt)
            nc.sync.dma_start(bst_sb[:], bst[:].bitcast(mm_dt))
            xq0 = None
            if main:
                xq0 = xq_pool.tile([128, NB, SC], mm_dt, name="xq", tag="xq")
                nc.sync.dma_start(xq0[:], xT[0, 0].bitcast(mm_dt))

            # W[l].T resident; chunk-0 compute paces these 1MB loads
            wt_sb = wt_pool.tile([128, NB, DOUT], mm_dt)
            wt_view = wT.rearrange("(n p) o -> p n o", p=128)
            for n in range(NB):
                nc.sync.dma_start(wt_sb[:, n, :],
                                  wt_view[:, n, :].bitcast(mm_dt))

            if not losses:
                losses_sb0 = small_pool.tile([1, 2], F32)
                nc.vector.memset(losses_sb0[:], 0.0)
                nc.sync.dma_start(losses_dram[:], losses_sb0[:])

            for j in range(2 if main else 0):
                for sc in range(nsc):
                    if j == 0 and sc == 0:
                        xq = xq0
                    else:
                        xq = xq_pool.tile([128, NB, SC], mm_dt, name="xq",
                                          tag="xq")
                        nc.sync.dma_start(xq[:], xT[j, sc].bitcast(mm_dt))
                    # uT[r2, s] = Am @ x_chunk  (accumulate over i-blocks)
                    pu = pu_pool.tile([R2, SC], F32)
                    for n in range(nb):
                        nc.tensor.matmul(
                            pu[:], amt_sb[:, n, j, :], xq[:, n, :],
                            start=(n == 0), stop=(n == nb - 1),
                        )
                    ut = ut_pool.tile([R2, SC], mm_dt)
                    nc.vector.tensor_copy(ut[:], pu[:])

                    for sb in range(NSB):
                        s0 = sb * 128
                        po_tiles = [po_pool.tile([128, OC], F32, name="po",
                                                 tag="po")
                                    for _ in range(NOC)]
                        for n in range(nb):
                            lhs = xq[:, n, s0:s0 + 128]
                            for oc in range(NOC):
                                nc.tensor.matmul(
                                    po_tiles[oc][:], lhs,
                                    wt_sb[:, n, oc * OC:(oc + 1) * OC],
                                    start=(n == 0), stop=False,
                                )
                        for oc in range(NOC):
                            nc.tensor.matmul(
                                po_tiles[oc][:], ut[:, s0:s0 + 128],
                                bst_sb[:, j, oc * OC:(oc + 1) * OC],
                                start=False, stop=True,
                            )
                        row = sc * SC + s0
                        for oc in range(NOC):
                            osb = osb_pool.tile([128, OC], F32)
                            nc.vector.tensor_copy(osb[:], po_tiles[oc][:])
                            nc.sync.dma_start(
                                out[j, row:row + 128, oc * OC:(oc + 1) * OC],
                                osb[:],
                            )
                    # emit losses after the first chunk: their tiny DMAs and
                    # DVE/ACT ops hide under the main stream
                    if losses and j == 0 and sc == 0:
                        _emit_losses(nc, small_pool, masks, scores,
                                     losses_dram)
            if losses and not main:
                _emit_losses(nc, small_pool, masks, scores, losses_dram)
    nc.compile()
    return nc


def _host_prep(x, W, A_w, B_w, topk_scores, neuron_masks, all_scores,
               topk_indices):
    """Shard + lay out inputs per core (transposes/gathers only)."""
    x = np.asarray(x, dtype=np.float32)
    W = np.asarray(W, dtype=np.float32)
    A_w = np.asarray(A_w, dtype=np.float32)
    B_w = np.asarray(B_w, dtype=np.float32)
    topk_scores = np.asarray(topk_scores, dtype=np.float32)
    neuron_masks = np.asarray(neuron_masks, dtype=np.float32)
    all_scores = np.ascontiguousarray(np.asarray(all_scores, dtype=np.float32))
    topk_indices = np.asarray(topk_indices).astype(np.int64)

    masks2d = np.ascontiguousarray(neuron_masks.reshape(1, E * B * R))
    wT_by_l = [np.ascontiguousarray(W[l].T) for l in range(L)]

    in_maps = []
    for c in range(N_CORES):
        l = c // 2
        bs = (2 * (c % 2), 2 * (c % 2) + 1)
        # chunk-major transposed x: [b][sc][p][n][s_in_chunk]
        xTc = np.ascontiguousarray(
            x[l, list(bs)].reshape(2, NSC, SC, NB, 128)
            .transpose(0, 1, 4, 3, 2)
        )
        amt = np.empty((128, NB, 2, R2), np.float32)
        bstm = np.empty((R2, 2, DOUT), np.float32)
        for jj, b in enumerate(bs):
            idx = topk_indices[b]                                   # [K]
            Am = (A_w[idx, l] * neuron_masks[idx, b][:, :, None])   # [K,R,DIN]
            amt[:, :, jj, :] = (
                Am.reshape(R2, DIN).T.reshape(NB, 128, R2).transpose(1, 0, 2)
            )
            Bs = B_w[idx, l] * topk_scores[b][:, None, None]        # [K,DOUT,R]
            bstm[:, jj, :] = Bs.transpose(0, 2, 1).reshape(R2, DOUT)
        in_maps.append({
            "xT": xTc,
            "wT": wT_by_l[l],
            "amt": np.ascontiguousarray(amt),
            "bst": np.ascontiguousarray(bstm),
            "masks": masks2d,
            "scores": all_scores,
        })
    return in_maps


def kernel(x, W, A_w, B_w, topk_scores, neuron_masks, all_scores,
           topk_indices):
    global _NC_CACHE, LAST_RESULT
    if _NC_CACHE is None:
        _NC_CACHE = _build()
    nc = _NC_CACHE

    in_maps = _host_prep(x, W, A_w, B_w, topk_scores, neuron_masks,
                         all_scores, topk_indices)
    res = run_bass_kernel_spmd(nc, in_maps, core_ids=list(range(N_CORES)))
    LAST_RESULT = res

    out = np.empty((L, B, S, DOUT), np.float32)
    for c in range(N_CORES):
        l = c // 2
        for jj, b in enumerate((2 * (c % 2), 2 * (c % 2) + 1)):
            out[l, b] = res.results[c]["out"][jj]
    sparsity_loss = np.float32(res.results[0]["losses"][0, 0])
    balance_loss = np.float32(res.results[0]["losses"][0, 1])
    return out, sparsity_loss, balance_loss


# revision 9
# speedup vs baseline: 1.0017x; 1.0017x over previous
"""ComposedLoRAModel forward on 8 TRN2 NeuronCores.

Math (per layer l, sample b):
    out[l,b]   = x[l,b] @ W[l].T + (x[l,b] @ Am_b.T) @ Bs_b.T
where Am_b [32, d_in] are the top-k adapters' A rows masked per-sample and
Bs_b [32, d_out] the score-scaled B columns -- the [d_out, d_in] delta of the
reference is never materialized, only its rank-32 factors are applied.
Plus two scalar losses (mask sparsity, softmax-balance KL), computed on
device without touching the tensor engine (it stays fp32r-only end to end;
mixing plain-fp32 matmuls into the fp32r stream stalls the PE on TRN2).

Sharding: data-parallel over the 16 (l, b) pairs -> core c owns layer c//2
and samples {2*(c%2), 2*(c%2)+1}. Each core holds W[l].T resident in SBUF
(16 MB), streams x.T in 2 MB chunks, and accumulates base + low-rank lora
contributions in PSUM using float32r (full-rate fp32 tensor-engine mode,
~1.5e-4 rel err). The first x chunk is DMA'd before W so the PE starts
within ~10us; chunk-0 compute then paces W's streaming arrival.

The host side only reshapes/transposes/gathers (no FLOPs on the big
tensors); every matmul, reduction and the loss math run on device.
"""
import numpy as np

import concourse.bacc as bacc
import concourse.mybir as mybir
import concourse.tile as tile
from concourse.bass_utils import run_bass_kernel_spmd

E, L, R, DIN, DOUT, B, K, S = 16, 4, 16, 2048, 2048, 4, 2, 2048
N_CORES = 8
R2 = K * R            # 32 combined low-rank columns
SC = 256              # s-chunk width streamed per DMA
NB = DIN // 128       # 16 contraction blocks
NSC = S // SC         # 8 s-chunks
NSB = SC // 128       # 2 128-row blocks per chunk
OC = 512              # output chunk width (one PSUM bank)
NOC = DOUT // OC      # 4 output chunks

F32 = mybir.dt.float32
F32R = mybir.dt.float32r

_NC_CACHE = None
LAST_RESULT = None  # stashed BassKernelResults for external profiling


def _emit_losses(nc, pool, masks, scores, losses_dram):
    """Sparsity mean + balance KL on DVE/ACT only (no tensor engine)."""
    losses_sb = pool.tile([1, 2], F32, name="losses_sb")

    masks_sb = pool.tile([1, E * B * R], F32, name="masks_sb")
    nc.sync.dma_start(masks_sb[:], masks[:])
    msum = pool.tile([1, 1], F32, name="msum")
    nc.vector.reduce_sum(msum[:], masks_sb[:], axis=mybir.AxisListType.X)
    nc.scalar.mul(losses_sb[:, 0:1], msum[:], 1.0 / (E * B * R))

    sc_sb = pool.tile([B, E], F32, name="sc_sb")
    nc.sync.dma_start(sc_sb[:], scores[:])
    rmax = pool.tile([B, 1], F32, name="rmax")
    nc.vector.reduce_max(rmax[:], sc_sb[:], axis=mybir.AxisListType.X)
    shifted = pool.tile([B, E], F32, name="shifted")
    nc.vector.tensor_scalar_sub(shifted[:], sc_sb[:], rmax[:])
    ex = pool.tile([B, E], F32, name="ex")
    nc.scalar.activation(ex[:], shifted[:], mybir.ActivationFunctionType.Exp)
    rsum = pool.tile([B, 1], F32, name="rsum")
    nc.vector.reduce_sum(rsum[:], ex[:], axis=mybir.AxisListType.X)
    rrec = pool.tile([B, 1], F32, name="rrec")
    nc.vector.reciprocal(rrec[:], rsum[:])
    probs = pool.tile([B, E], F32, name="probs")
    nc.vector.tensor_scalar_mul(probs[:], ex[:], rrec[:])
    # flatten probs onto one partition; batch-mean via strided view reduce
    pflat = pool.tile([1, B, E], F32, name="pflat")
    nc.sync.dma_start(pflat[:], probs[:])
    mp_row = pool.tile([1, E], F32, name="mp_row")
    nc.vector.reduce_sum(mp_row[:], pflat[:].transpose([0, 2, 1]),
                         axis=mybir.AxisListType.X)
    logp = pool.tile([1, E], F32, name="logp")
    # ln(mean) = ln(sum * 1/B) via the activation pre-scale
    nc.scalar.activation(logp[:], mp_row[:], mybir.ActivationFunctionType.Ln,
                         scale=float(1.0 / B))
    u = 1.0 / E
    t_sb = pool.tile([1, E], F32, name="t_sb")
    nc.scalar.mul(t_sb[:], logp[:], float(-u / E))
    nc.vector.tensor_scalar_add(t_sb[:], t_sb[:], float(u * np.log(u) / E))
    bal = pool.tile([1, 1], F32, name="bal")
    nc.vector.reduce_sum(bal[:], t_sb[:], axis=mybir.AxisListType.X)
    nc.vector.tensor_copy(losses_sb[:, 1:2], bal[:])
    nc.sync.dma_start(losses_dram[:], losses_sb[:])


def _build(losses=True, main=True, mm_dt=F32R, nb=NB, nsc=NSC):
    nc = bacc.Bacc("TRN2", target_bir_lowering=False, debug=False,
                   num_devices=N_CORES)
    # x, pre-transposed and chunk-major on host: [b][sc][p][n][s_in_chunk]
    xT = nc.dram_tensor("xT", [2, NSC, 128, NB, SC], F32,
                        kind="ExternalInput")
    wT = nc.dram_tensor("wT", [DIN, DOUT], F32, kind="ExternalInput")
    amt = nc.dram_tensor("amt", [128, NB, 2, R2], F32, kind="ExternalInput")
    bst = nc.dram_tensor("bst", [R2, 2, DOUT], F32, kind="ExternalInput")
    masks = nc.dram_tensor("masks", [1, E * B * R], F32, kind="ExternalInput")
    scores = nc.dram_tensor("scores", [B, E], F32, kind="ExternalInput")
    out = nc.dram_tensor("out", [2, S, DOUT], F32, kind="ExternalOutput")
    losses_dram = nc.dram_tensor("losses", [1, 2], F32, kind="ExternalOutput")

    with tile.TileContext(nc) as tc:
        with (
            tc.tile_pool(name="wt", bufs=1) as wt_pool,
            tc.tile_pool(name="xq", bufs=2) as xq_pool,
            tc.tile_pool(name="small", bufs=1) as small_pool,
            tc.tile_pool(name="ut", bufs=2) as ut_pool,
            tc.tile_pool(name="osb", bufs=4) as osb_pool,
            tc.tile_pool(name="pu", bufs=2, space="PSUM") as pu_pool,
            tc.tile_pool(name="po", bufs=6, space="PSUM") as po_pool,
        ):
            # adapter factors + first x chunk first: they gate the PE start
            amt_sb = small_pool.tile([128, NB, 2, R2], mm_dt)
            nc.sync.dma_start(amt_sb[:], amt[:].bitcast(mm_dt))
            xq0 = None
            if main:
                # first chunk loaded per-i-block so uT matmuls start on the
                # first 128KB instead of waiting for the whole 2MB
                xq0 = xq_pool.tile([128, NB, SC], mm_dt, name="xq", tag="xq")
                for n in range(NB):
                    nc.sync.dma_start(xq0[:, n, :],
                                      xT[0, 0, :, n, :].bitcast(mm_dt))

            # W[l].T resident; chunk-0 compute paces these 1MB loads
            wt_sb = wt_pool.tile([128, NB, DOUT], mm_dt)
            wt_view = wT.rearrange("(n p) o -> p n o", p=128)
            for n in range(NB):
                nc.sync.dma_start(wt_sb[:, n, :],
                                  wt_view[:, n, :].bitcast(mm_dt))
            # bst is only needed by chunk-0's trailing lora matmul
            bst_sb = small_pool.tile([R2, 2, DOUT], mm_dt)
            nc.sync.dma_start(bst_sb[:], bst[:].bitcast(mm_dt))

            if not losses:
                losses_sb0 = small_pool.tile([1, 2], F32)
                nc.vector.memset(losses_sb0[:], 0.0)
                nc.sync.dma_start(losses_dram[:], losses_sb0[:])

            for j in range(2 if main else 0):
                for sc in range(nsc):
                    if j == 0 and sc == 0:
                        xq = xq0
                    else:
                        xq = xq_pool.tile([128, NB, SC], mm_dt, name="xq",
                                          tag="xq")
                        nc.sync.dma_start(xq[:], xT[j, sc].bitcast(mm_dt))
                    # uT[r2, s] = Am @ x_chunk  (accumulate over i-blocks)
                    pu = pu_pool.tile([R2, SC], F32)
                    for n in range(nb):
                        nc.tensor.matmul(
                            pu[:], amt_sb[:, n, j, :], xq[:, n, :],
                            start=(n == 0), stop=(n == nb - 1),
                        )
                    ut = ut_pool.tile([R2, SC], mm_dt)
                    nc.vector.tensor_copy(ut[:], pu[:])

                    for sb in range(NSB):
                        s0 = sb * 128
                        po_tiles = [po_pool.tile([128, OC], F32, name="po",
                                                 tag="po")
                                    for _ in range(NOC)]
                        for n in range(nb):
                            lhs = xq[:, n, s0:s0 + 128]
                            for oc in range(NOC):
                                nc.tensor.matmul(
                                    po_tiles[oc][:], lhs,
                                    wt_sb[:, n, oc * OC:(oc + 1) * OC],
                                    start=(n == 0), stop=False,
                                )
                        for oc in range(NOC):
                            nc.tensor.matmul(
                                po_tiles[oc][:], ut[:, s0:s0 + 128],
                                bst_sb[:, j, oc * OC:(oc + 1) * OC],
                                start=False, stop=True,
                            )
                        row = sc * SC + s0
                        for oc in range(NOC):
                            osb = osb_pool.tile([128, OC], F32)
                            nc.vector.tensor_copy(osb[:], po_tiles[oc][:])
                            nc.sync.dma_start(
                                out[j, row:row + 128, oc * OC:(oc + 1) * OC],
                                osb[:],
                            )
                    # emit losses after the first chunk: their tiny DMAs and
                    # DVE/ACT ops hide under the main stream
                    if losses and j == 0 and sc == 0:
                        _emit_losses(nc, small_pool, masks, scores,
                                     losses_dram)
            if losses and not main:
                _emit_losses(nc, small_pool, masks, scores, losses_dram)
    nc.compile()
    return nc


def _host_prep(x, W, A_w, B_w, topk_scores, neuron_masks, all_scores,
               topk_indices):
    """Shard + lay out inputs per core (transposes/gathers only)."""
    x = np.asarray(x, dtype=np.float32)
    W = np.asarray(W, dtype=np.float32)
    A_w = np.asarray(A_w, dtype=np.float32)
    B_w = np.asarray(B_w, dtype=np.float32)
    topk_scores = np.asarray(topk_scores, dtype=np.float32)
    neuron_masks = np.asarray(neuron_masks, dtype=np.float32)
    all_scores = np.ascontiguousarray(np.asarray(all_scores, dtype=np.float32))
    topk_indices = np.asarray(topk_indices).astype(np.int64)

    masks2d = np.ascontiguousarray(neuron_masks.reshape(1, E * B * R))
    wT_by_l = [np.ascontiguousarray(W[l].T) for l in range(L)]

    in_maps = []
    for c in range(N_CORES):
        l = c // 2
        bs = (2 * (c % 2), 2 * (c % 2) + 1)
        # chunk-major transposed x: [b][sc][p][n][s_in_chunk]
        xTc = np.ascontiguousarray(
            x[l, list(bs)].reshape(2, NSC, SC, NB, 128)
            .transpose(0, 1, 4, 3, 2)
        )
        amt = np.empty((128, NB, 2, R2), np.float32)
        bstm = np.empty((R2, 2, DOUT), np.float32)
        for jj, b in enumerate(bs):
            idx = topk_indices[b]                                   # [K]
            Am = (A_w[idx, l] * neuron_masks[idx, b][:, :, None])   # [K,R,DIN]
            amt[:, :, jj, :] = (
                Am.reshape(R2, DIN).T.reshape(NB, 128, R2).transpose(1, 0, 2)
            )
            Bs = B_w[idx, l] * topk_scores[b][:, None, None]        # [K,DOUT,R]
            bstm[:, jj, :] = Bs.transpose(0, 2, 1).reshape(R2, DOUT)
        in_maps.append({
            "xT": xTc,
            "wT": wT_by_l[l],
            "amt": np.ascontiguousarray(amt),
            "bst": np.ascontiguousarray(bstm),
            "masks": masks2d,
            "scores": all_scores,
        })
    return in_maps


def kernel(x, W, A_w, B_w, topk_scores, neuron_masks, all_scores,
           topk_indices):
    global _NC_CACHE, LAST_RESULT
    if _NC_CACHE is None:
        _NC_CACHE = _build()
    nc = _NC_CACHE

    in_maps = _host_prep(x, W, A_w, B_w, topk_scores, neuron_masks,
                         all_scores, topk_indices)
    res = run_bass_kernel_spmd(nc, in_maps, core_ids=list(range(N_CORES)))
    LAST_RESULT = res

    out = np.empty((L, B, S, DOUT), np.float32)
    for c in range(N_CORES):
        l = c // 2
        for jj, b in enumerate((2 * (c % 2), 2 * (c % 2) + 1)):
            out[l, b] = res.results[c]["out"][jj]
    sparsity_loss = np.float32(res.results[0]["losses"][0, 0])
    balance_loss = np.float32(res.results[0]["losses"][0, 1])
    return out, sparsity_loss, balance_loss


# revision 10
# speedup vs baseline: 1.0200x; 1.0182x over previous
"""ComposedLoRAModel forward on 8 TRN2 NeuronCores.

Math (per layer l, sample b):
    out[l,b]   = x[l,b] @ W[l].T + (x[l,b] @ Am_b.T) @ Bs_b.T
where Am_b [32, d_in] are the top-k adapters' A rows masked per-sample and
Bs_b [32, d_out] the score-scaled B columns -- the [d_out, d_in] delta of the
reference is never materialized, only its rank-32 factors are applied.
Plus two scalar losses (mask sparsity, softmax-balance KL), computed on
device without touching the tensor engine (it stays fp32r-only end to end;
mixing plain-fp32 matmuls into the fp32r stream stalls the PE on TRN2).

Sharding: data-parallel over the 16 (l, b) pairs -> core c owns layer c//2
and samples {2*(c%2), 2*(c%2)+1}. Each core holds W[l].T resident in SBUF
(16 MB), streams x.T in 2 MB chunks, and accumulates base + low-rank lora
contributions in PSUM using float32r (full-rate fp32 tensor-engine mode,
~1.5e-4 rel err). The first x chunk is DMA'd before W so the PE starts
within ~10us; chunk-0 compute then paces W's streaming arrival.

The host side only reshapes/transposes/gathers (no FLOPs on the big
tensors); every matmul, reduction and the loss math run on device.
"""
import numpy as np

import concourse.bacc as bacc
import concourse.mybir as mybir
import concourse.tile as tile
from concourse.bass_utils import run_bass_kernel_spmd

E, L, R, DIN, DOUT, B, K, S = 16, 4, 16, 2048, 2048, 4, 2, 2048
N_CORES = 8
R2 = K * R            # 32 combined low-rank columns
SC = 256              # s-chunk width streamed per DMA
NB = DIN // 128       # 16 contraction blocks
NSC = S // SC         # 8 s-chunks
NSB = SC // 128       # 2 128-row blocks per chunk
OC = 512              # output chunk width (one PSUM bank)
NOC = DOUT // OC      # 4 output chunks

F32 = mybir.dt.float32
F32R = mybir.dt.float32r

_NC_CACHE = None
LAST_RESULT = None  # stashed BassKernelResults for external profiling


def _emit_losses(nc, pool, masks, scores, losses_dram):
    """Sparsity mean + balance KL on DVE/ACT only (no tensor engine)."""
    losses_sb = pool.tile([1, 2], F32, name="losses_sb")

    masks_sb = pool.tile([1, E * B * R], F32, name="masks_sb")
    nc.sync.dma_start(masks_sb[:], masks[:])
    msum = pool.tile([1, 1], F32, name="msum")
    nc.vector.reduce_sum(msum[:], masks_sb[:], axis=mybir.AxisListType.X)
    nc.scalar.mul(losses_sb[:, 0:1], msum[:], 1.0 / (E * B * R))

    sc_sb = pool.tile([B, E], F32, name="sc_sb")
    nc.sync.dma_start(sc_sb[:], scores[:])
    rmax = pool.tile([B, 1], F32, name="rmax")
    nc.vector.reduce_max(rmax[:], sc_sb[:], axis=mybir.AxisListType.X)
    shifted = pool.tile([B, E], F32, name="shifted")
    nc.vector.tensor_scalar_sub(shifted[:], sc_sb[:], rmax[:])
    ex = pool.tile([B, E], F32, name="ex")
    nc.scalar.activation(ex[:], shifted[:], mybir.ActivationFunctionType.Exp)
    rsum = pool.tile([B, 1], F32, name="rsum")
    nc.vector.reduce_sum(rsum[:], ex[:], axis=mybir.AxisListType.X)
    rrec = pool.tile([B, 1], F32, name="rrec")
    nc.vector.reciprocal(rrec[:], rsum[:])
    probs = pool.tile([B, E], F32, name="probs")
    nc.vector.tensor_scalar_mul(probs[:], ex[:], rrec[:])
    # flatten probs onto one partition; batch-mean via strided view reduce
    pflat = pool.tile([1, B, E], F32, name="pflat")
    nc.sync.dma_start(pflat[:], probs[:])
    mp_row = pool.tile([1, E], F32, name="mp_row")
    nc.vector.reduce_sum(mp_row[:], pflat[:].transpose([0, 2, 1]),
                         axis=mybir.AxisListType.X)
    logp = pool.tile([1, E], F32, name="logp")
    # ln(mean) = ln(sum * 1/B) via the activation pre-scale
    nc.scalar.activation(logp[:], mp_row[:], mybir.ActivationFunctionType.Ln,
                         scale=float(1.0 / B))
    u = 1.0 / E
    t_sb = pool.tile([1, E], F32, name="t_sb")
    nc.scalar.mul(t_sb[:], logp[:], float(-u / E))
    nc.vector.tensor_scalar_add(t_sb[:], t_sb[:], float(u * np.log(u) / E))
    bal = pool.tile([1, 1], F32, name="bal")
    nc.vector.reduce_sum(bal[:], t_sb[:], axis=mybir.AxisListType.X)
    nc.vector.tensor_copy(losses_sb[:, 1:2], bal[:])
    nc.sync.dma_start(losses_dram[:], losses_sb[:])


def _build(losses=True, main=True, mm_dt=F32R, nb=NB, nsc=NSC):
    nc = bacc.Bacc("TRN2", target_bir_lowering=False, debug=False,
                   num_devices=N_CORES)
    # x, pre-transposed and chunk-major on host: [b][sc][p][n][s_in_chunk]
    xT = nc.dram_tensor("xT", [2, NSC, 128, NB, SC], F32,
                        kind="ExternalInput")
    wT = nc.dram_tensor("wT", [DIN, DOUT], F32, kind="ExternalInput")
    amt = nc.dram_tensor("amt", [128, NB, 2, R2], F32, kind="ExternalInput")
    bst = nc.dram_tensor("bst", [R2, 2, DOUT], F32, kind="ExternalInput")
    masks = nc.dram_tensor("masks", [1, E * B * R], F32, kind="ExternalInput")
    scores = nc.dram_tensor("scores", [B, E], F32, kind="ExternalInput")
    out = nc.dram_tensor("out", [2, S, DOUT], F32, kind="ExternalOutput")
    losses_dram = nc.dram_tensor("losses", [1, 2], F32, kind="ExternalOutput")

    with tile.TileContext(nc) as tc:
        with (
            tc.tile_pool(name="wt", bufs=1) as wt_pool,
            tc.tile_pool(name="xq", bufs=2) as xq_pool,
            tc.tile_pool(name="small", bufs=1) as small_pool,
            tc.tile_pool(name="ut", bufs=2) as ut_pool,
            tc.tile_pool(name="osb", bufs=4) as osb_pool,
            tc.tile_pool(name="pu", bufs=2, space="PSUM") as pu_pool,
            tc.tile_pool(name="po", bufs=6, space="PSUM") as po_pool,
        ):
            # adapter factors + first x chunk first: they gate the PE start
            amt_sb = small_pool.tile([128, NB, 2, R2], mm_dt)
            nc.sync.dma_start(amt_sb[:], amt[:].bitcast(mm_dt))
            xq_map = {}
            if main:
                # first chunk loaded per-i-block so uT matmuls start on the
                # first 128KB instead of waiting for the whole 2MB
                xq0 = xq_pool.tile([128, NB, SC], mm_dt, name="xq", tag="xq")
                for n in range(NB):
                    nc.sync.dma_start(xq0[:, n, :],
                                      xT[0, 0, :, n, :].bitcast(mm_dt))
                xq_map[(0, 0)] = xq0
            # bst is needed by chunk-0's lora matmul at ~15us
            bst_sb = small_pool.tile([R2, 2, DOUT], mm_dt)
            nc.sync.dma_start(bst_sb[:], bst[:].bitcast(mm_dt))

            # W[l].T resident, streamed o-half-major (oc01 fully before oc23)
            # so the first two chunks' oc01 passes can run during the oc23
            # load; second x chunk queued between the halves
            wt_sb = wt_pool.tile([128, NB, DOUT], mm_dt)
            wt_view = wT.rearrange("(n p) o -> p n o", p=128)
            HO = DOUT // 2
            for n in range(NB):
                nc.sync.dma_start(wt_sb[:, n, 0:HO],
                                  wt_view[:, n, 0:HO].bitcast(mm_dt))
            if main and nsc > 1:
                xq1 = xq_pool.tile([128, NB, SC], mm_dt, name="xq", tag="xq")
                nc.sync.dma_start(xq1[:], xT[0, 1].bitcast(mm_dt))
                xq_map[(0, 1)] = xq1
            for n in range(NB):
                nc.sync.dma_start(wt_sb[:, n, HO:DOUT],
                                  wt_view[:, n, HO:DOUT].bitcast(mm_dt))

            if not losses:
                losses_sb0 = small_pool.tile([1, 2], F32)
                nc.vector.memset(losses_sb0[:], 0.0)
                nc.sync.dma_start(losses_dram[:], losses_sb0[:])

            ut_map = {}

            def do_ut(j, sc):
                xq = xq_map[(j, sc)]
                pu = pu_pool.tile([R2, SC], F32, name="pu", tag="pu")
                for n in range(nb):
                    nc.tensor.matmul(
                        pu[:], amt_sb[:, n, j, :], xq[:, n, :],
                        start=(n == 0), stop=(n == nb - 1),
                    )
                ut = ut_pool.tile([R2, SC], mm_dt, name="ut", tag="ut")
                nc.vector.tensor_copy(ut[:], pu[:])
                ut_map[(j, sc)] = ut

            def do_ocs(j, sc, ocs):
                xq = xq_map[(j, sc)]
                ut = ut_map[(j, sc)]
                for sb in range(NSB):
                    s0 = sb * 128
                    po_tiles = [po_pool.tile([128, OC], F32, name="po",
                                             tag="po") for _ in ocs]
                    for n in range(nb):
                        lhs = xq[:, n, s0:s0 + 128]
                        for i, oc in enumerate(ocs):
                            nc.tensor.matmul(
                                po_tiles[i][:], lhs,
                                wt_sb[:, n, oc * OC:(oc + 1) * OC],
                                start=(n == 0), stop=False,
                            )
                    for i, oc in enumerate(ocs):
                        nc.tensor.matmul(
                            po_tiles[i][:], ut[:, s0:s0 + 128],
                            bst_sb[:, j, oc * OC:(oc + 1) * OC],
                            start=False, stop=True,
                        )
                    row = sc * SC + s0
                    for i, oc in enumerate(ocs):
                        osb = osb_pool.tile([128, OC], F32, name="osb",
                                            tag="osb")
                        nc.vector.tensor_copy(osb[:], po_tiles[i][:])
                        nc.sync.dma_start(
                            out[j, row:row + 128, oc * OC:(oc + 1) * OC],
                            osb[:],
                        )

            if main:
                all_chunks = [(j, sc) for j in range(2) for sc in range(nsc)]
                special = [c for c in ((0, 0), (0, 1)) if c in all_chunks]
                # wavefront: oc01 of the first two chunks runs while W-oc23
                # still streams, then their oc23, then steady state
                for c in special:
                    do_ut(*c)
                    do_ocs(*c, ocs=(0, 1))
                    if c == special[0] and losses:
                        _emit_losses(nc, small_pool, masks, scores,
                                     losses_dram)
                for c in special:
                    do_ocs(*c, ocs=(2, 3))
                for c in all_chunks:
                    if c in special:
                        continue
                    j, sc = c
                    xq = xq_pool.tile([128, NB, SC], mm_dt, name="xq",
                                      tag="xq")
                    nc.sync.dma_start(xq[:], xT[j, sc].bitcast(mm_dt))
                    xq_map[c] = xq
                    do_ut(j, sc)
                    do_ocs(j, sc, ocs=(0, 1, 2, 3))
            if losses and not main:
                _emit_losses(nc, small_pool, masks, scores, losses_dram)
    nc.compile()
    return nc


def _host_prep(x, W, A_w, B_w, topk_scores, neuron_masks, all_scores,
               topk_indices):
    """Shard + lay out inputs per core (transposes/gathers only)."""
    x = np.asarray(x, dtype=np.float32)
    W = np.asarray(W, dtype=np.float32)
    A_w = np.asarray(A_w, dtype=np.float32)
    B_w = np.asarray(B_w, dtype=np.float32)
    topk_scores = np.asarray(topk_scores, dtype=np.float32)
    neuron_masks = np.asarray(neuron_masks, dtype=np.float32)
    all_scores = np.ascontiguousarray(np.asarray(all_scores, dtype=np.float32))
    topk_indices = np.asarray(topk_indices).astype(np.int64)

    masks2d = np.ascontiguousarray(neuron_masks.reshape(1, E * B * R))
    wT_by_l = [np.ascontiguousarray(W[l].T) for l in range(L)]

    in_maps = []
    for c in range(N_CORES):
        l = c // 2
        bs = (2 * (c % 2), 2 * (c % 2) + 1)
        # chunk-major transposed x: [b][sc][p][n][s_in_chunk]
        xTc = np.ascontiguousarray(
            x[l, list(bs)].reshape(2, NSC, SC, NB, 128)
            .transpose(0, 1, 4, 3, 2)
        )
        amt = np.empty((128, NB, 2, R2), np.float32)
        bstm = np.empty((R2, 2, DOUT), np.float32)
        for jj, b in enumerate(bs):
            idx = topk_indices[b]                                   # [K]
            Am = (A_w[idx, l] * neuron_masks[idx, b][:, :, None])   # [K,R,DIN]
            amt[:, :, jj, :] = (
                Am.reshape(R2, DIN).T.reshape(NB, 128, R2).transpose(1, 0, 2)
            )
            Bs = B_w[idx, l] * topk_scores[b][:, None, None]        # [K,DOUT,R]
            bstm[:, jj, :] = Bs.transpose(0, 2, 1).reshape(R2, DOUT)
        in_maps.append({
            "xT": xTc,
            "wT": wT_by_l[l],
            "amt": np.ascontiguousarray(amt),
            "bst": np.ascontiguousarray(bstm),
            "masks": masks2d,
            "scores": all_scores,
        })
    return in_maps


def kernel(x, W, A_w, B_w, topk_scores, neuron_masks, all_scores,
           topk_indices):
    global _NC_CACHE, LAST_RESULT
    if _NC_CACHE is None:
        _NC_CACHE = _build()
    nc = _NC_CACHE

    in_maps = _host_prep(x, W, A_w, B_w, topk_scores, neuron_masks,
                         all_scores, topk_indices)
    res = run_bass_kernel_spmd(nc, in_maps, core_ids=list(range(N_CORES)))
    LAST_RESULT = res

    out = np.empty((L, B, S, DOUT), np.float32)
    for c in range(N_CORES):
        l = c // 2
        for jj, b in enumerate((2 * (c % 2), 2 * (c % 2) + 1)):
            out[l, b] = res.results[c]["out"][jj]
    sparsity_loss = np.float32(res.results[0]["losses"][0, 0])
    balance_loss = np.float32(res.results[0]["losses"][0, 1])
    return out, sparsity_loss, balance_loss
